# revision 6
# baseline (speedup 1.0000x reference)
"""Trainium2 Bass kernel for a cascade of 4 biquad IIR sections (DF2T).

Approach: the cascaded IIR filter is LTI with an impulse response that decays
below fp32 noise within ~32 taps (max pole modulus ~0.49 for the given
coefficient scaling).  We therefore evaluate it as an exact-to-fp32 truncated
FIR with K_TAPS=64 taps, expressed as TensorE matmuls against a pair of
128x128 Toeplitz band matrices built on the host from the (tiny) coefficient
inputs.

Layout: the input (B=512, T=32768) is transposed on the host to (T, B) so
time lies on SBUF partitions and batch on the free dim.  Output chunk q
(128 consecutive timesteps x 512 batch) is:

    y[q*128 + i, :] = sum_j h[j] * X[(q+1)*128 + i - j, :]      (X has a
                      128-row halo of history prepended)
                    = W1.T @ xtile[q] + W0.T @ xtile[q+1]

with W0[k, i] = h[i - k]        (0 <= i-k < K_TAPS)
     W1[k, i] = h[128 + i - k]  (0 <= 128+i-k < K_TAPS)

Sharding: time is split across the 8 cores (4096 steps each + 128-row halo
from the previous shard; zeros for core 0, matching zero initial state).
Batch stays whole (512 free dim = full PSUM bank per matmul).
"""

import numpy as np
from contextlib import ExitStack

import concourse.bass as bass
import concourse.tile as tile
from concourse import bacc, mybir
from concourse.bass_utils import run_bass_kernel_spmd

B = 512
T = 32768
NCORES = 8
T_LOC = T // NCORES            # 4096
HALO = 128
K_TAPS = 64
N_SECTIONS = 4
IN_ROWS = HALO + T_LOC         # 4224
N_IN_TILES = IN_ROWS // 128    # 33
N_CHUNKS = T_LOC // 128        # 32
IN_BATCH = 4                   # input tiles per dma_start (1 MiB)
OUT_BATCH = 4                  # output chunks per dma_start (1 MiB)

MM_DT = mybir.dt.float32       # exact fp32 matmul (float32r is ~2^-12 precise)

LAST_RESULTS = None            # BassKernelResults of the most recent run
_NC_CACHE = {}


def _impulse_response(b, a, n):
    """First n taps of the cascaded DF2T biquad impulse response (float64)."""
    b = np.asarray(b, np.float64)
    a = np.asarray(a, np.float64)
    sig = np.zeros(n, np.float64)
    sig[0] = 1.0
    for k in range(N_SECTIONS):
        y = np.zeros(n, np.float64)
        s1 = 0.0
        s2 = 0.0
        for t in range(n):
            u = sig[t]
            yt = b[k, 0] * u + s1
            s1 = b[k, 1] * u - a[k, 0] * yt + s2
            s2 = b[k, 2] * u - a[k, 1] * yt
            y[t] = yt
        sig = y
    return sig


def _toeplitz_weights(b, a):
    h = _impulse_response(b, a, K_TAPS)
    k = np.arange(128)[:, None]
    i = np.arange(128)[None, :]
    j0 = i - k
    w0 = np.where((j0 >= 0) & (j0 < K_TAPS), h[np.clip(j0, 0, K_TAPS - 1)], 0.0)
    j1 = 128 + i - k
    w1 = np.where((j1 >= 0) & (j1 < K_TAPS), h[np.clip(j1, 0, K_TAPS - 1)], 0.0)
    return w0.astype(np.float32), w1.astype(np.float32)


def _build_nc(mm_dt):
    nc = bacc.Bacc(
        "TRN2", target_bir_lowering=False, debug=False, num_devices=NCORES
    )
    xin = nc.dram_tensor("xin", [IN_ROWS, B], mm_dt, kind="ExternalInput").ap()
    w0 = nc.dram_tensor("w0", [128, 128], mm_dt, kind="ExternalInput").ap()
    w1 = nc.dram_tensor("w1", [128, 128], mm_dt, kind="ExternalInput").ap()
    yout = nc.dram_tensor(
        "yout", [T_LOC, B], mybir.dt.float32, kind="ExternalOutput"
    ).ap()

    # fp32/f32r matmuls lower to a self-loading LDWEIGHTS with a single
    # sync-wait slot in walrus codegen, so no matmul may carry more than one
    # semaphore wait.  Every tile a matmul reads is therefore staged through a
    # VectorE copy: matmuls then only ever wait on the (single) DVE
    # processor semaphore, and the DVE copies absorb the per-queue DMA sems.
    with tile.TileContext(nc) as tc, ExitStack() as ctx:
        n_in_batches = (N_IN_TILES + IN_BATCH - 1) // IN_BATCH
        n_out_bufs = (N_CHUNKS + OUT_BATCH - 1) // OUT_BATCH
        wstage = ctx.enter_context(tc.tile_pool(name="wstage", bufs=2))
        wpool = ctx.enter_context(tc.tile_pool(name="w", bufs=2))
        stagepool = ctx.enter_context(tc.tile_pool(name="xstage", bufs=3))
        inpool = ctx.enter_context(tc.tile_pool(name="xbuf", bufs=n_in_batches))
        pspool = ctx.enter_context(tc.tile_pool(name="ps", bufs=8, space="PSUM"))
        outpool = ctx.enter_context(tc.tile_pool(name="ybuf", bufs=n_out_bufs))

        wts = []
        for wap in (w0, w1):
            ws = wstage.tile([128, 128], mm_dt, tag="wstage")
            nc.sync.dma_start(ws[:], wap)
            wt = wpool.tile([128, 128], mm_dt, tag="w")
            nc.vector.tensor_copy(wt[:], ws[:])
            wts.append(wt)
        w0t, w1t = wts

        in_bufs = []
        for bi in range(0, N_IN_TILES, IN_BATCH):
            n = min(IN_BATCH, N_IN_TILES - bi)
            st = stagepool.tile([128, n * B], mm_dt, tag="xstage")
            nc.sync.dma_start(
                st.rearrange("p (n b) -> p n b", b=B),
                xin[bi * 128 : (bi + n) * 128, :].rearrange("(n p) b -> p n b", p=128),
            )
            t = inpool.tile([128, n * B], mm_dt, tag="xbuf")
            nc.vector.tensor_copy(t[:], st[:])
            in_bufs.append(t)

        def xtile(j):
            t = in_bufs[j // IN_BATCH]
            o = (j % IN_BATCH) * B
            return t[:, o : o + B]

        for bo in range(0, N_CHUNKS, OUT_BATCH):
            n = min(OUT_BATCH, N_CHUNKS - bo)
            ot = outpool.tile([128, n * B], mybir.dt.float32, tag="ybuf")
            for qi in range(n):
                q = bo + qi
                pt = pspool.tile([128, B], mybir.dt.float32, tag="ps")
                nc.tensor.matmul(pt[:], w1t[:], xtile(q), start=True, stop=False)
                nc.tensor.matmul(pt[:], w0t[:], xtile(q + 1), start=False, stop=True)
                nc.vector.tensor_copy(ot[:, qi * B : (qi + 1) * B], pt[:])
            nc.sync.dma_start(
                yout[bo * 128 : (bo + n) * 128, :].rearrange("(n p) b -> p n b", p=128),
                ot.rearrange("p (n b) -> p n b", b=B),
            )
    nc.compile()
    return nc


def _get_nc():
    if MM_DT not in _NC_CACHE:
        _NC_CACHE[MM_DT] = _build_nc(MM_DT)
    return _NC_CACHE[MM_DT]


def kernel(x, b, a):
    global LAST_RESULTS
    x = np.asarray(x, np.float32)
    assert x.shape == (B, T, 1), x.shape

    xt = np.ascontiguousarray(x[:, :, 0].T)                        # (T, B)
    xpad = np.concatenate([np.zeros((HALO, B), np.float32), xt], axis=0)
    w0, w1 = _toeplitz_weights(b, a)

    in_maps = [
        {
            "xin": np.ascontiguousarray(xpad[c * T_LOC : c * T_LOC + IN_ROWS]),
            "w0": w0,
            "w1": w1,
        }
        for c in range(NCORES)
    ]
    res = run_bass_kernel_spmd(_get_nc(), in_maps, list(range(NCORES)))
    LAST_RESULTS = res
    yt = np.concatenate([res.results[c]["yout"] for c in range(NCORES)], axis=0)
    return np.ascontiguousarray(yt.T)[:, :, None]


# revision 8
# speedup vs baseline: 1.0500x; 1.0500x over previous
"""Trainium2 Bass kernel for a cascade of 4 biquad IIR sections (DF2T).

Approach: the cascaded IIR filter is LTI with an impulse response that decays
below fp32 noise within ~32 taps (max pole modulus ~0.49 for the given
coefficient scaling).  We therefore evaluate it as an exact-to-fp32 truncated
FIR with K_TAPS=64 taps, expressed as TensorE matmuls against a pair of
128x128 Toeplitz band matrices built on the host from the (tiny) coefficient
inputs.

Layout: the input (B=512, T=32768) is transposed on the host to (T, B) so
time lies on SBUF partitions and batch on the free dim.  Output chunk q
(128 consecutive timesteps x 512 batch) is:

    y[q*128 + i, :] = sum_j h[j] * X[(q+1)*128 + i - j, :]      (X has a
                      128-row halo of history prepended)
                    = W1.T @ xtile[q] + W0.T @ xtile[q+1]

with W0[k, i] = h[i - k]        (0 <= i-k < K_TAPS)
     W1[k, i] = h[128 + i - k]  (0 <= 128+i-k < K_TAPS)

Sharding: time is split across the 8 cores (4096 steps each + 128-row halo
from the previous shard; zeros for core 0, matching zero initial state).
Batch stays whole (512 free dim = full PSUM bank per matmul).
"""

import numpy as np
from contextlib import ExitStack

import concourse.bass as bass
import concourse.tile as tile
from concourse import bacc, mybir
from concourse.bass_utils import run_bass_kernel_spmd

B = 512
T = 32768
NCORES = 8
T_LOC = T // NCORES            # 4096
HALO = 128
K_TAPS = 64
N_SECTIONS = 4
IN_ROWS = HALO + T_LOC         # 4224
N_IN_TILES = IN_ROWS // 128    # 33
N_CHUNKS = T_LOC // 128        # 32
IN_BATCH = 4                   # input tiles per dma_start (1 MiB)
OUT_BATCH = 4                  # output chunks per dma_start (1 MiB)

MM_DT = mybir.dt.float32       # exact fp32 matmul (float32r is ~2^-12 precise)

LAST_RESULTS = None            # BassKernelResults of the most recent run
_NC_CACHE = {}


def _impulse_response(b, a, n):
    """First n taps of the cascaded DF2T biquad impulse response (float64)."""
    b = np.asarray(b, np.float64)
    a = np.asarray(a, np.float64)
    sig = np.zeros(n, np.float64)
    sig[0] = 1.0
    for k in range(N_SECTIONS):
        y = np.zeros(n, np.float64)
        s1 = 0.0
        s2 = 0.0
        for t in range(n):
            u = sig[t]
            yt = b[k, 0] * u + s1
            s1 = b[k, 1] * u - a[k, 0] * yt + s2
            s2 = b[k, 2] * u - a[k, 1] * yt
            y[t] = yt
        sig = y
    return sig


def _toeplitz_weights(b, a):
    h = _impulse_response(b, a, K_TAPS)
    k = np.arange(128)[:, None]
    i = np.arange(128)[None, :]
    j0 = i - k
    w0 = np.where((j0 >= 0) & (j0 < K_TAPS), h[np.clip(j0, 0, K_TAPS - 1)], 0.0)
    j1 = 128 + i - k
    w1 = np.where((j1 >= 0) & (j1 < K_TAPS), h[np.clip(j1, 0, K_TAPS - 1)], 0.0)
    return w0.astype(np.float32), w1.astype(np.float32)


def _build_nc(mm_dt):
    nc = bacc.Bacc(
        "TRN2", target_bir_lowering=False, debug=False, num_devices=NCORES
    )
    xin = nc.dram_tensor("xin", [IN_ROWS, B], mm_dt, kind="ExternalInput").ap()
    w0 = nc.dram_tensor("w0", [128, 128], mm_dt, kind="ExternalInput").ap()
    w1 = nc.dram_tensor("w1", [128, 128], mm_dt, kind="ExternalInput").ap()
    yout = nc.dram_tensor(
        "yout", [T_LOC, B], mybir.dt.float32, kind="ExternalOutput"
    ).ap()

    # Note: fp32/f32r matmuls lower to a self-loading LDWEIGHTS with a single
    # sync-wait slot in walrus codegen; Bacc's compile() legalizes any
    # multi-wait instruction by hoisting extra waits into event semaphores.
    with tile.TileContext(nc) as tc, ExitStack() as ctx:
        # input batch sizes: small leading batches so the first matmuls can
        # start as soon as possible, then 1 MiB batches for DMA efficiency
        in_batches = []
        bi = 0
        for sz in (1, 1, 2):
            in_batches.append((bi, sz))
            bi += sz
        while bi < N_IN_TILES:
            sz = min(IN_BATCH, N_IN_TILES - bi)
            in_batches.append((bi, sz))
            bi += sz

        n_out_bufs = (N_CHUNKS + OUT_BATCH - 1) // OUT_BATCH
        wpool = ctx.enter_context(tc.tile_pool(name="w", bufs=2))
        inpool = ctx.enter_context(tc.tile_pool(name="xbuf", bufs=len(in_batches)))
        pspool = ctx.enter_context(tc.tile_pool(name="ps", bufs=8, space="PSUM"))
        outpool = ctx.enter_context(tc.tile_pool(name="ybuf", bufs=n_out_bufs))

        w0t = wpool.tile([128, 128], mm_dt, tag="w")
        nc.sync.dma_start(w0t[:], w0)
        w1t = wpool.tile([128, 128], mm_dt, tag="w")
        nc.sync.dma_start(w1t[:], w1)

        tile_of = {}
        for start, n in in_batches:
            t = inpool.tile([128, n * B], mm_dt, tag="xbuf")
            nc.sync.dma_start(
                t.rearrange("p (n b) -> p n b", b=B),
                xin[start * 128 : (start + n) * 128, :].rearrange(
                    "(n p) b -> p n b", p=128
                ),
            )
            for j in range(start, start + n):
                tile_of[j] = t[:, (j - start) * B : (j - start + 1) * B]

        for bo in range(0, N_CHUNKS, OUT_BATCH):
            n = min(OUT_BATCH, N_CHUNKS - bo)
            ot = outpool.tile([128, n * B], mybir.dt.float32, tag="ybuf")
            for qi in range(n):
                q = bo + qi
                pt = pspool.tile([128, B], mybir.dt.float32, tag="ps")
                nc.tensor.matmul(pt[:], w1t[:], tile_of[q], start=True, stop=False)
                nc.tensor.matmul(pt[:], w0t[:], tile_of[q + 1], start=False, stop=True)
                # alternate PSUM-drain between DVE and ACT so the copy stage
                # keeps up with the matmul stream on either engine
                if q % 2 == 0:
                    nc.vector.tensor_copy(ot[:, qi * B : (qi + 1) * B], pt[:])
                else:
                    nc.scalar.copy(ot[:, qi * B : (qi + 1) * B], pt[:])
            nc.sync.dma_start(
                yout[bo * 128 : (bo + n) * 128, :].rearrange("(n p) b -> p n b", p=128),
                ot.rearrange("p (n b) -> p n b", b=B),
            )
    nc.compile()
    return nc


def _get_nc():
    if MM_DT not in _NC_CACHE:
        _NC_CACHE[MM_DT] = _build_nc(MM_DT)
    return _NC_CACHE[MM_DT]


def kernel(x, b, a):
    global LAST_RESULTS
    x = np.asarray(x, np.float32)
    assert x.shape == (B, T, 1), x.shape

    xt = np.ascontiguousarray(x[:, :, 0].T)                        # (T, B)
    xpad = np.concatenate([np.zeros((HALO, B), np.float32), xt], axis=0)
    w0, w1 = _toeplitz_weights(b, a)

    in_maps = [
        {
            "xin": np.ascontiguousarray(xpad[c * T_LOC : c * T_LOC + IN_ROWS]),
            "w0": w0,
            "w1": w1,
        }
        for c in range(NCORES)
    ]
    res = run_bass_kernel_spmd(_get_nc(), in_maps, list(range(NCORES)))
    LAST_RESULTS = res
    yt = np.concatenate([res.results[c]["yout"] for c in range(NCORES)], axis=0)
    return np.ascontiguousarray(yt.T)[:, :, None]


# revision 10
# speedup vs baseline: 1.0971x; 1.0448x over previous
"""Trainium2 Bass kernel for a cascade of 4 biquad IIR sections (DF2T).

Approach: the cascaded IIR filter is LTI with an impulse response that decays
below fp32 noise within ~32 taps (max pole modulus ~0.49 for the given
coefficient scaling).  We therefore evaluate it as an exact-to-fp32 truncated
FIR with K_TAPS=64 taps, expressed as TensorE matmuls against a pair of
128x128 Toeplitz band matrices built on the host from the (tiny) coefficient
inputs.

Layout: the input (B=512, T=32768) is transposed on the host to (T, B) so
time lies on SBUF partitions and batch on the free dim.  Output chunk q
(128 consecutive timesteps x 512 batch) is:

    y[q*128 + i, :] = sum_j h[j] * X[(q+1)*128 + i - j, :]      (X has a
                      128-row halo of history prepended)
                    = W1.T @ xtile[q] + W0.T @ xtile[q+1]

with W0[k, i] = h[i - k]        (0 <= i-k < K_TAPS)
     W1[k, i] = h[128 + i - k]  (0 <= 128+i-k < K_TAPS)

Sharding: time is split across the 8 cores (4096 steps each + 128-row halo
from the previous shard; zeros for core 0, matching zero initial state).
Batch stays whole (512 free dim = full PSUM bank per matmul).
"""

import numpy as np
from contextlib import ExitStack

import concourse.bass as bass
import concourse.tile as tile
from concourse import bacc, mybir
from concourse.bass_utils import run_bass_kernel_spmd

B = 512
T = 32768
NCORES = 8
T_LOC = T // NCORES            # 4096
HALO = 128
K_TAPS = 64
N_SECTIONS = 4
IN_ROWS = HALO + T_LOC         # 4224
N_IN_TILES = IN_ROWS // 128    # 33
N_CHUNKS = T_LOC // 128        # 32
IN_BATCH = 4                   # input tiles per dma_start (1 MiB)
OUT_BATCH = 4                  # output chunks per dma_start (1 MiB)

MM_DT = mybir.dt.float32       # exact fp32 matmul (float32r is ~2^-12 precise)

LAST_RESULTS = None            # BassKernelResults of the most recent run
_NC_CACHE = {}


def _impulse_response(b, a, n):
    """First n taps of the cascaded DF2T biquad impulse response (float64)."""
    b = np.asarray(b, np.float64)
    a = np.asarray(a, np.float64)
    sig = np.zeros(n, np.float64)
    sig[0] = 1.0
    for k in range(N_SECTIONS):
        y = np.zeros(n, np.float64)
        s1 = 0.0
        s2 = 0.0
        for t in range(n):
            u = sig[t]
            yt = b[k, 0] * u + s1
            s1 = b[k, 1] * u - a[k, 0] * yt + s2
            s2 = b[k, 2] * u - a[k, 1] * yt
            y[t] = yt
        sig = y
    return sig


def _toeplitz_weights(b, a):
    h = _impulse_response(b, a, K_TAPS)
    k = np.arange(128)[:, None]
    i = np.arange(128)[None, :]
    j0 = i - k
    w0 = np.where((j0 >= 0) & (j0 < K_TAPS), h[np.clip(j0, 0, K_TAPS - 1)], 0.0)
    j1 = 128 + i - k
    w1 = np.where((j1 >= 0) & (j1 < K_TAPS), h[np.clip(j1, 0, K_TAPS - 1)], 0.0)
    return w0.astype(np.float32), w1.astype(np.float32)


def _build_nc(mm_dt):
    nc = bacc.Bacc(
        "TRN2", target_bir_lowering=False, debug=False, num_devices=NCORES
    )
    xin = nc.dram_tensor("xin", [IN_ROWS, B], mm_dt, kind="ExternalInput").ap()
    w0 = nc.dram_tensor("w0", [128, 128], mm_dt, kind="ExternalInput").ap()
    w1 = nc.dram_tensor("w1", [128, 128], mm_dt, kind="ExternalInput").ap()
    yout = nc.dram_tensor(
        "yout", [T_LOC, B], mybir.dt.float32, kind="ExternalOutput"
    ).ap()

    # Note: fp32/f32r matmuls lower to a self-loading LDWEIGHTS with a single
    # sync-wait slot in walrus codegen; Bacc's compile() legalizes any
    # multi-wait instruction by hoisting extra waits into event semaphores.
    with tile.TileContext(nc) as tc, ExitStack() as ctx:
        # input batch sizes: small leading batches so the first matmuls can
        # start as soon as possible, then 1 MiB batches for DMA efficiency
        in_batches = []
        bi = 0
        for sz in (1, 1, 2):
            in_batches.append((bi, sz))
            bi += sz
        while bi < N_IN_TILES:
            sz = min(IN_BATCH, N_IN_TILES - bi)
            in_batches.append((bi, sz))
            bi += sz

        # output batches: steady 1 MiB, ramp down at the end so the final
        # (serial) store after the last matmul is small
        out_batches = []
        bo = 0
        while bo < N_CHUNKS - 4:
            out_batches.append((bo, OUT_BATCH))
            bo += OUT_BATCH
        for sz in (2, 1, 1):
            out_batches.append((bo, sz))
            bo += sz

        wpool = ctx.enter_context(tc.tile_pool(name="w", bufs=2))
        warmpool = ctx.enter_context(tc.tile_pool(name="warm", bufs=2))
        inpool = ctx.enter_context(tc.tile_pool(name="xbuf", bufs=len(in_batches)))
        pspool = ctx.enter_context(tc.tile_pool(name="ps", bufs=8, space="PSUM"))
        outpool = ctx.enter_context(tc.tile_pool(name="ybuf", bufs=len(out_batches)))

        w0t = wpool.tile([128, 128], mm_dt, tag="w")
        nc.sync.dma_start(w0t[:], w0)
        w1t = wpool.tile([128, 128], mm_dt, tag="w")
        nc.sync.dma_start(w1t[:], w1)

        # HAM warm-up: the real matmuls only start once the first input DMAs
        # land (~10us in); keep the PE busy before that with dummy bf16
        # matmuls on a memset tile so the clock gate is at 2.4 GHz (and the
        # ~3.4us warm-up window is already paid) when real work begins.
        warm_in = warmpool.tile([128, 512], mybir.dt.bfloat16, tag="warm_in")
        nc.gpsimd.memset(warm_in[:], 0.0)
        warm_ps = pspool.tile([128, 512], mybir.dt.float32, tag="ps")
        for _ in range(16):
            nc.tensor.matmul(
                warm_ps[:], warm_in[:, :128], warm_in[:], start=True, stop=True
            )

        tile_of = {}
        for start, n in in_batches:
            t = inpool.tile([128, n * B], mm_dt, tag="xbuf")
            nc.sync.dma_start(
                t.rearrange("p (n b) -> p n b", b=B),
                xin[start * 128 : (start + n) * 128, :].rearrange(
                    "(n p) b -> p n b", p=128
                ),
            )
            for j in range(start, start + n):
                tile_of[j] = t[:, (j - start) * B : (j - start + 1) * B]

        for bo, n in out_batches:
            ot = outpool.tile([128, n * B], mybir.dt.float32, tag="ybuf")
            for qi in range(n):
                q = bo + qi
                pt = pspool.tile([128, B], mybir.dt.float32, tag="ps")
                nc.tensor.matmul(pt[:], w1t[:], tile_of[q], start=True, stop=False)
                nc.tensor.matmul(pt[:], w0t[:], tile_of[q + 1], start=False, stop=True)
                # alternate PSUM-drain between DVE and ACT so the copy stage
                # keeps up with the matmul stream on either engine
                if q % 2 == 0:
                    nc.vector.tensor_copy(ot[:, qi * B : (qi + 1) * B], pt[:])
                else:
                    nc.scalar.copy(ot[:, qi * B : (qi + 1) * B], pt[:])
            nc.sync.dma_start(
                yout[bo * 128 : (bo + n) * 128, :].rearrange("(n p) b -> p n b", p=128),
                ot.rearrange("p (n b) -> p n b", b=B),
            )
    nc.compile()
    return nc


def _get_nc():
    if MM_DT not in _NC_CACHE:
        _NC_CACHE[MM_DT] = _build_nc(MM_DT)
    return _NC_CACHE[MM_DT]


def kernel(x, b, a):
    global LAST_RESULTS
    x = np.asarray(x, np.float32)
    assert x.shape == (B, T, 1), x.shape

    xt = np.ascontiguousarray(x[:, :, 0].T)                        # (T, B)
    xpad = np.concatenate([np.zeros((HALO, B), np.float32), xt], axis=0)
    w0, w1 = _toeplitz_weights(b, a)

    in_maps = [
        {
            "xin": np.ascontiguousarray(xpad[c * T_LOC : c * T_LOC + IN_ROWS]),
            "w0": w0,
            "w1": w1,
        }
        for c in range(NCORES)
    ]
    res = run_bass_kernel_spmd(_get_nc(), in_maps, list(range(NCORES)))
    LAST_RESULTS = res
    yt = np.concatenate([res.results[c]["yout"] for c in range(NCORES)], axis=0)
    return np.ascontiguousarray(yt.T)[:, :, None]


# revision 11
# speedup vs baseline: 1.2975x; 1.1827x over previous
"""Trainium2 Bass kernel for a cascade of 4 biquad IIR sections (DF2T).

Approach: the cascaded IIR filter is LTI with an impulse response that decays
below fp32 noise within ~32 taps (max pole modulus ~0.49 for the given
coefficient scaling).  We therefore evaluate it as an exact-to-fp32 truncated
FIR with K_TAPS=64 taps, expressed as TensorE matmuls against a pair of
128x128 Toeplitz band matrices built on the host from the (tiny) coefficient
inputs.

Layout: the input (B=512, T=32768) is transposed on the host to (T, B) so
time lies on SBUF partitions and batch on the free dim.  Output chunk q
(128 consecutive timesteps x 512 batch) is:

    y[q*128 + i, :] = sum_j h[j] * X[(q+1)*128 + i - j, :]      (X has a
                      128-row halo of history prepended)
                    = W1.T @ xtile[q] + W0.T @ xtile[q+1]

with W0[k, i] = h[i - k]        (0 <= i-k < K_TAPS)
     W1[k, i] = h[128 + i - k]  (0 <= 128+i-k < K_TAPS)

Precision/speed: MODE
  - "bf16x3" (default): x and W are split on the host into bf16 (hi, lo)
    pairs; each W.T @ x is computed as Wh@xh + Wh@xl + Wl@xh (the Wl@xl term
    is ~2^-16 relative and dropped).  bf16 matmuls run at 1 cycle/row vs 4
    for fp32, products are exact in fp32 PSUM; measured absmax error vs the
    fp64 reference is ~9e-6 of scale.  6 matmuls per output chunk.
  - "fp32": exact fp32 matmuls (4 cycles/row), absmax error ~4e-7 of scale.
    2 matmuls per output chunk, ~35% slower end-to-end.

Sharding: time is split across the 8 cores (4096 steps each + 128-row halo
from the previous shard; zeros for core 0, matching zero initial state).
Batch stays whole (512 free dim = one full PSUM bank per matmul).
"""

import os
import numpy as np
from contextlib import ExitStack

import ml_dtypes

import concourse.bass as bass
import concourse.tile as tile
from concourse import bacc, mybir
from concourse.bass_utils import run_bass_kernel_spmd

B = 512
T = 32768
NCORES = 8
T_LOC = T // NCORES            # 4096
HALO = 128
K_TAPS = 64
N_SECTIONS = 4
IN_ROWS = HALO + T_LOC         # 4224
N_IN_TILES = IN_ROWS // 128    # 33
N_CHUNKS = T_LOC // 128        # 32
IN_BATCH = 4                   # input tiles per dma_start
OUT_BATCH = 4                  # output chunks per dma_start (1 MiB)

MODE = os.environ.get("KERNEL_MODE", "bf16x3")   # "bf16x3" | "fp32"

LAST_RESULTS = None            # BassKernelResults of the most recent run
_NC_CACHE = {}


def _impulse_response(b, a, n):
    """First n taps of the cascaded DF2T biquad impulse response (float64)."""
    b = np.asarray(b, np.float64)
    a = np.asarray(a, np.float64)
    sig = np.zeros(n, np.float64)
    sig[0] = 1.0
    for k in range(N_SECTIONS):
        y = np.zeros(n, np.float64)
        s1 = 0.0
        s2 = 0.0
        for t in range(n):
            u = sig[t]
            yt = b[k, 0] * u + s1
            s1 = b[k, 1] * u - a[k, 0] * yt + s2
            s2 = b[k, 2] * u - a[k, 1] * yt
            y[t] = yt
        sig = y
    return sig


def _toeplitz_weights(b, a):
    h = _impulse_response(b, a, K_TAPS)
    k = np.arange(128)[:, None]
    i = np.arange(128)[None, :]
    j0 = i - k
    w0 = np.where((j0 >= 0) & (j0 < K_TAPS), h[np.clip(j0, 0, K_TAPS - 1)], 0.0)
    j1 = 128 + i - k
    w1 = np.where((j1 >= 0) & (j1 < K_TAPS), h[np.clip(j1, 0, K_TAPS - 1)], 0.0)
    return w0.astype(np.float32), w1.astype(np.float32)


def _split_bf16(v):
    hi = v.astype(ml_dtypes.bfloat16)
    lo = (v - hi.astype(np.float32)).astype(ml_dtypes.bfloat16)
    return hi, lo


def _in_out_batches():
    # small leading input batches so the first matmuls start early, then
    # steady batches; output ramps down so the final store is small
    in_batches = []
    bi = 0
    for sz in (1, 1, 2):
        in_batches.append((bi, sz))
        bi += sz
    while bi < N_IN_TILES:
        sz = min(IN_BATCH, N_IN_TILES - bi)
        in_batches.append((bi, sz))
        bi += sz
    out_batches = []
    bo = 0
    while bo < N_CHUNKS - 4:
        out_batches.append((bo, OUT_BATCH))
        bo += OUT_BATCH
    for sz in (2, 1, 1):
        out_batches.append((bo, sz))
        bo += sz
    return in_batches, out_batches


def _build_nc(mode):
    nc = bacc.Bacc(
        "TRN2", target_bir_lowering=False, debug=False, num_devices=NCORES
    )
    f32 = mybir.dt.float32
    bf16 = mybir.dt.bfloat16
    yout = nc.dram_tensor("yout", [T_LOC, B], f32, kind="ExternalOutput").ap()
    if mode == "fp32":
        xin = nc.dram_tensor("xin", [IN_ROWS, B], f32, kind="ExternalInput").ap()
        w0 = nc.dram_tensor("w0", [128, 128], f32, kind="ExternalInput").ap()
        w1 = nc.dram_tensor("w1", [128, 128], f32, kind="ExternalInput").ap()
    else:
        # hi/lo bf16 split of x (leading dim 2) and of each weight matrix
        xin = nc.dram_tensor("xin", [2, IN_ROWS, B], bf16, kind="ExternalInput").ap()
        w0 = nc.dram_tensor("w0", [2, 128, 128], bf16, kind="ExternalInput").ap()
        w1 = nc.dram_tensor("w1", [2, 128, 128], bf16, kind="ExternalInput").ap()

    in_batches, out_batches = _in_out_batches()

    # Note: fp32 matmuls lower to a self-loading LDWEIGHTS with a single
    # sync-wait slot in walrus codegen; Bacc's compile() legalizes any
    # multi-wait instruction by hoisting extra waits into event semaphores.
    with tile.TileContext(nc) as tc, ExitStack() as ctx:
        wpool = ctx.enter_context(tc.tile_pool(name="w", bufs=4))
        warmpool = ctx.enter_context(tc.tile_pool(name="warm", bufs=2))
        inpool = ctx.enter_context(
            tc.tile_pool(name="xbuf", bufs=2 * len(in_batches))
        )
        pspool = ctx.enter_context(tc.tile_pool(name="ps", bufs=8, space="PSUM"))
        outpool = ctx.enter_context(tc.tile_pool(name="ybuf", bufs=len(out_batches)))

        mm_dt = f32 if mode == "fp32" else bf16

        # weight tiles
        if mode == "fp32":
            w0t = wpool.tile([128, 128], f32, tag="w")
            nc.sync.dma_start(w0t[:], w0)
            w1t = wpool.tile([128, 128], f32, tag="w")
            nc.sync.dma_start(w1t[:], w1)
        else:
            wtiles = {}
            for nm, wap in (("w0", w0), ("w1", w1)):
                for s in range(2):
                    t = wpool.tile([128, 128], bf16, tag="w")
                    nc.sync.dma_start(t[:], wap[s])
                    wtiles[(nm, s)] = t

        # HAM warm-up: the real matmuls only start once the first input DMAs
        # land (~8us in); keep the PE busy before that with dummy bf16
        # matmuls on a memset tile so the clock gate is at 2.4 GHz (and the
        # ~3.4us warm-up window already paid) when real work begins.
        warm_in = warmpool.tile([128, 512], bf16, tag="warm_in")
        nc.gpsimd.memset(warm_in[:], 0.0)
        warm_ps = pspool.tile([128, 512], f32, tag="ps")
        for _ in range(16):
            nc.tensor.matmul(
                warm_ps[:], warm_in[:, :128], warm_in[:], start=True, stop=True
            )

        # input tiles
        tile_of = {}   # (split, j) -> AP;  fp32 mode uses split=0 only
        splits = (0,) if mode == "fp32" else (0, 1)
        for start, n in in_batches:
            for s in splits:
                t = inpool.tile([128, n * B], mm_dt, tag="xbuf")
                src = (
                    xin[start * 128 : (start + n) * 128, :]
                    if mode == "fp32"
                    else xin[s, start * 128 : (start + n) * 128, :]
                )
                nc.sync.dma_start(
                    t.rearrange("p (n b) -> p n b", b=B),
                    src.rearrange("(n p) b -> p n b", p=128),
                )
                for j in range(start, start + n):
                    tile_of[(s, j)] = t[:, (j - start) * B : (j - start + 1) * B]

        for bo, n in out_batches:
            ot = outpool.tile([128, n * B], f32, tag="ybuf")
            for qi in range(n):
                q = bo + qi
                pt = pspool.tile([128, B], f32, tag="ps")
                if mode == "fp32":
                    nc.tensor.matmul(
                        pt[:], w1t[:], tile_of[(0, q)], start=True, stop=False
                    )
                    nc.tensor.matmul(
                        pt[:], w0t[:], tile_of[(0, q + 1)], start=False, stop=True
                    )
                else:
                    terms = [
                        (wtiles[("w1", 0)], tile_of[(0, q)]),
                        (wtiles[("w1", 1)], tile_of[(0, q)]),
                        (wtiles[("w1", 0)], tile_of[(1, q)]),
                        (wtiles[("w0", 0)], tile_of[(0, q + 1)]),
                        (wtiles[("w0", 1)], tile_of[(0, q + 1)]),
                        (wtiles[("w0", 0)], tile_of[(1, q + 1)]),
                    ]
                    for ti, (wt, xt) in enumerate(terms):
                        nc.tensor.matmul(
                            pt[:], wt[:], xt,
                            start=(ti == 0), stop=(ti == len(terms) - 1),
                        )
                # alternate PSUM-drain between DVE and ACT so the copy stage
                # keeps up with the matmul stream on either engine
                if q % 2 == 0:
                    nc.vector.tensor_copy(ot[:, qi * B : (qi + 1) * B], pt[:])
                else:
                    nc.scalar.copy(ot[:, qi * B : (qi + 1) * B], pt[:])
            nc.sync.dma_start(
                yout[bo * 128 : (bo + n) * 128, :].rearrange("(n p) b -> p n b", p=128),
                ot.rearrange("p (n b) -> p n b", b=B),
            )
    nc.compile()
    return nc


def _get_nc(mode):
    if mode not in _NC_CACHE:
        _NC_CACHE[mode] = _build_nc(mode)
    return _NC_CACHE[mode]


def kernel(x, b, a):
    global LAST_RESULTS
    x = np.asarray(x, np.float32)
    assert x.shape == (B, T, 1), x.shape

    xt = np.ascontiguousarray(x[:, :, 0].T)                        # (T, B)
    xpad = np.concatenate([np.zeros((HALO, B), np.float32), xt], axis=0)
    w0f, w1f = _toeplitz_weights(b, a)

    if MODE == "fp32":
        in_maps = [
            {
                "xin": np.ascontiguousarray(xpad[c * T_LOC : c * T_LOC + IN_ROWS]),
                "w0": w0f,
                "w1": w1f,
            }
            for c in range(NCORES)
        ]
    else:
        xh, xl = _split_bf16(xpad)
        w0s = np.stack(_split_bf16(w0f))
        w1s = np.stack(_split_bf16(w1f))
        in_maps = [
            {
                "xin": np.ascontiguousarray(
                    np.stack(
                        [
                            xh[c * T_LOC : c * T_LOC + IN_ROWS],
                            xl[c * T_LOC : c * T_LOC + IN_ROWS],
                        ]
                    )
                ),
                "w0": w0s,
                "w1": w1s,
            }
            for c in range(NCORES)
        ]

    res = run_bass_kernel_spmd(_get_nc(MODE), in_maps, list(range(NCORES)))
    LAST_RESULTS = res
    yt = np.concatenate([res.results[c]["yout"] for c in range(NCORES)], axis=0)
    return np.ascontiguousarray(yt.T)[:, :, None]
